# revision 18
# baseline (speedup 1.0000x reference)
"""Trainium2 Bass kernel for nn_Candidate_Scorer.

Reference computation:
    b = G_p @ wb            # [N,1]
    e = G_p @ we            # [N,1]
    num = exp(b + e.T)      # [N,N]
    den = sum(num)
    P = triu(num / den)
    top_k(P.reshape(-1), k) -> ((row, col) indices, values)

Key structure exploited:
  * num = exp(b) * exp(e).T is rank-1, so den = sum(exp(b)) * sum(exp(e)).
    No N x N reduction is needed.
  * exp is monotone, so the top-k of exp(b_i + e_j) over {j >= i} is the
    top-k of b_i + e_j over the same set -- selected from the two
    N-vectors with an exact thresholding argument (see _select_topk).

Device work (SPMD over 8 cores, rows sharded): b = G@wb, e = G@we as
batched multiply + reduce on the Vector engine (rows on partitions -
avoids the PE fp32 4-cycles/row penalty, keeps full f32 accuracy); the
weight vectors are broadcast across partitions by the otherwise-idle
TensorEngine (ones x w -> PSUM); exp on the Scalar engine; softmax-
denominator partials via a final Vector reduce.  Raw engine programs
(bacc) with manual semaphores; input G is pipelined over three DMA
channels (sync HWDGE, scalar HWDGE, gpsimd SWDGE) block by block so
compute starts after ~100 KB lands.
Host work (gather/merge): concatenate shards, exact top-k candidate
selection from the N-vectors, final value/index assembly.
"""

import numpy as np

N = 8192
D = 200
N_CORES = 8
ROWS = N // N_CORES    # 1024 rows per core
BLK = ROWS // 128      # 8 row-blocks of 128 partitions

_COMPILED = {}


def _build_program():
    """Per-core SPMD program (bacc, manual sync).

    Inputs (per core):
      "w"   [1, 400]   wb then we
      "g1"  [128, 400] G blocks 0-1 (row blk*128+p at partition p)
      "g2"  [128, 600] G blocks 2-4
      "g3"  [128, 600] G blocks 5-7
    Outputs (per core):
      "out"  [128, 16] cols v*8+blk = b (v=0) / e (v=1) values
      "out2" [128, 2]  per-partition sums of exp(b), exp(e)
    """
    import concourse.bass as bass
    import concourse.bacc as bacc
    import concourse.mybir as mybir

    dt = mybir.dt.float32
    fexp = mybir.ActivationFunctionType.Exp
    nc = bacc.Bacc("TRN2", target_bir_lowering=False, debug=False,
                   num_devices=N_CORES)

    w_d = nc.dram_tensor("w", [1, 2 * D], dt, kind="ExternalInput")
    g1_d = nc.dram_tensor("g1", [128, 2 * D], dt, kind="ExternalInput")
    g2_d = nc.dram_tensor("g2", [128, 3 * D], dt, kind="ExternalInput")
    g3_d = nc.dram_tensor("g3", [128, 3 * D], dt, kind="ExternalInput")
    out_d = nc.dram_tensor("out", [128, 16], dt, kind="ExternalOutput")
    out2_d = nc.dram_tensor("out2", [128, 2], dt, kind="ExternalOutput")

    with (
        nc.sbuf_tensor("g_s", [128, BLK * D], dt) as g_s,
        nc.sbuf_tensor("w_s", [1, 2 * D], dt) as w_s,
        nc.sbuf_tensor("ones_s", [1, 128], dt) as ones_s,
        nc.sbuf_tensor("out_s", [128, 16], dt) as out_s,
        nc.sbuf_tensor("out2_s", [128, 2], dt) as out2_s,
        nc.sbuf_tensor("prod_s", [128, 2 * 3 * D], dt) as prod_s,
        nc.sbuf_tensor("ebe_s", [128, 2 * BLK], dt) as ebe_s,
        nc.psum_tensor("w_p", [128, 2 * D], dt) as w_p,
        nc.semaphore("s_w") as s_w,
        nc.semaphore("s_r1") as s_r1,
        nc.semaphore("s_r2") as s_r2,
        nc.semaphore("s_r3") as s_r3,
        nc.semaphore("s_on") as s_on,
        nc.semaphore("s_pe") as s_pe,
        nc.semaphore("s_dve") as s_dve,
        nc.semaphore("s_act") as s_act,
        nc.semaphore("s_done") as s_done,
        nc.semaphore("s_out") as s_out,
        nc.Block() as block,
    ):
        # w broadcast [128, 2, nb, 200] view over the PSUM copy
        def w4(nb):
            return (w_p[:].rearrange("p (v d) -> p v d", v=2)
                    .rearrange("p v (z d) -> p v z d", z=1)
                    .broadcast_to((128, 2, nb, D)))

        # out columns viewed as [2, BLK]
        bev = out_s[:].rearrange("p (v z) -> p v z", v=2)

        @block.sync
        def _(sync):
            sync.dma_start(w_s[:], w_d[:]).then_inc(s_w, 16)
            sync.dma_start(g_s[:, 0:D], g1_d[:, 0:D]).then_inc(s_r1, 16)
            sync.dma_start(g_s[:, D:2 * D], g1_d[:, D:]).then_inc(s_r1, 16)
            sync.wait_ge(s_dve, 1)
            sync.dma_start(out_d[:], out_s[:]).then_inc(s_out, 16)
            sync.wait_ge(s_done, 1)
            sync.dma_start(out2_d[:], out2_s[:]).then_inc(s_out, 16)
            sync.wait_ge(s_out, 32)

        @block.scalar
        def _(scalar):
            # blocks 2-4 on the ACT HWDGE ring, parallel with ring1
            scalar.dma_start(g_s[:, 2 * D:5 * D], g2_d[:]).then_inc(s_r2, 16)
            # warm the Exp table while the DMAs fly (result discarded)
            nc.scalar.activation(ebe_s[:, 0:1], out_s[:, 0:1], fexp)
            scalar.wait_ge(s_dve, 1)
            nc.scalar.activation(ebe_s[:], out_s[:], fexp).then_inc(s_act, 1)

        @block.gpsimd
        def _(gpsimd):
            nc.gpsimd.memset(ones_s[:], 1.0).then_inc(s_on, 1)
            # blocks 5-7 via SWDGE, parallel with both HWDGE rings
            gpsimd.dma_start(g_s[:, 5 * D:], g3_d[:]).then_inc(s_r3, 16)

        @block.tensor
        def _(tensor):
            # broadcast wb,we across partitions: ones.T @ w -> [128, 400]
            tensor.wait_ge(s_w, 16)
            tensor.wait_ge(s_on, 1)
            nc.tensor.matmul(w_p[:], ones_s[:], w_s[:],
                             start=True, stop=True).then_inc(s_pe, 1)

        @block.vector
        def _(vector):
            # (z0, nb, [(sem, threshold), ...]) data gates per chunk
            plan = [(0, 1, [(s_pe, 1), (s_r1, 16)]),
                    (1, 1, [(s_r1, 32)]),
                    (2, 3, [(s_r2, 16)]),
                    (5, 3, [(s_r3, 16)])]
            for z0, nb, gates in plan:
                for sem, thr in gates:
                    vector.wait_ge(sem, thr)
                g4 = (g_s[:, z0 * D:(z0 + nb) * D]
                      .rearrange("p (z d) -> p z d", z=nb)
                      .rearrange("p z (u d) -> p u z d", u=1)
                      .broadcast_to((128, 2, nb, D)))
                p4 = (prod_s[:, 0:2 * nb * D]
                      .rearrange("p (v z d) -> p v z d", v=2, z=nb))
                nc.vector.tensor_tensor(p4, g4, w4(nb),
                                        op=mybir.AluOpType.mult)
                ins = nc.vector.reduce_sum(bev[:, :, z0:z0 + nb], p4,
                                           axis=mybir.AxisListType.X)
            ins.then_inc(s_dve, 1)
            # softmax-denominator partials: per-partition sums of exp
            vector.wait_ge(s_act, 1)
            e3 = ebe_s[:].rearrange("p (v z) -> p v z", v=2)
            nc.vector.reduce_sum(out2_s[:], e3, axis=mybir.AxisListType.X
                                 ).then_inc(s_done, 1)

    nc.compile()
    return nc


def _get_program():
    if "nc" not in _COMPILED:
        _COMPILED["nc"] = _build_program()
    return _COMPILED["nc"]


def _pack_inputs(G_p, wb, we):
    w = np.concatenate([wb.reshape(-1), we.reshape(-1)]).astype(
        np.float32).reshape(1, 2 * D)
    in_maps = []
    for c in range(N_CORES):
        shard = G_p[c * ROWS:(c + 1) * ROWS, :].astype(np.float32)
        blocks = shard.reshape(BLK, 128, D).transpose(1, 0, 2)  # [128,8,200]
        in_maps.append({
            "w": w,
            "g1": np.ascontiguousarray(blocks[:, 0:2, :].reshape(128, 2 * D)),
            "g2": np.ascontiguousarray(blocks[:, 2:5, :].reshape(128, 3 * D)),
            "g3": np.ascontiguousarray(blocks[:, 5:8, :].reshape(128, 3 * D)),
        })
    return in_maps


def _run_device(G_p, wb, we, trace=False):
    from concourse.bass_utils import run_bass_kernel_spmd

    nc = _get_program()
    in_maps = _pack_inputs(G_p, wb, we)
    res = run_bass_kernel_spmd(nc, in_maps, core_ids=list(range(N_CORES)),
                               trace=trace)
    return res


def _select_topk(b, e, den, k):
    """Exact top-k of exp(b_i + e_j)/den over {(i, j): j >= i}.

    Threshold argument: rowbest[i] = b[i] + max(e[i:]) is each row's best
    pair value. The k-th largest rowbest T is a lower bound on the k-th
    largest pair value (k distinct rows each contain a pair >= T), so
    every true top-k pair has value >= T. We enumerate all valid pairs
    with b_i + e_j >= T (minus a small safety margin) and rank them
    exactly as jax.lax.top_k does: by f32 value descending, ties broken
    by lower flat index.
    """
    bf = b.astype(np.float32)
    ef = e.astype(np.float32)
    n = bf.shape[0]

    suff = np.maximum.accumulate(ef[::-1])[::-1]   # suffix max of e
    rowbest = bf + suff
    kth = np.partition(rowbest, n - k)[n - k] - np.float32(1e-4)

    order_e = np.lexsort((np.arange(n), -ef))
    e_sorted = ef[order_e]

    rows = np.where(rowbest >= kth)[0]
    cand_i, cand_j = [], []
    for i in rows:
        t = kth - bf[i]
        cnt = int(np.searchsorted(-e_sorted, -t, side="right"))
        if cnt == 0:
            continue
        js = order_e[:cnt]
        js = js[js >= i]
        if js.size:
            cand_i.append(np.full(js.size, i, dtype=np.int64))
            cand_j.append(js)
    ci = np.concatenate(cand_i)
    cj = np.concatenate(cand_j)

    # values exactly as the reference computes them: f32 add, f32 exp,
    # f32 divide
    s = (bf[ci] + ef[cj]).astype(np.float32)
    v = np.exp(s).astype(np.float32) / np.float32(den)
    flat = ci * n + cj
    order = np.lexsort((flat, -v))[:k]
    top_i = ci[order]
    top_j = cj[order]
    idx = np.stack([top_i, top_j], axis=1).astype(np.int32)
    return idx, v[order].astype(np.float32)


def kernel(G_p, wb, we, k):
    G_p = np.asarray(G_p, dtype=np.float32)
    wb = np.asarray(wb, dtype=np.float32).reshape(D, 1)
    we = np.asarray(we, dtype=np.float32).reshape(D, 1)
    k = int(k)

    res = _run_device(G_p, wb, we)
    outs = res.results

    # out[:, v*8+blk] at partition p = b/e[blk*128 + p]
    b = np.concatenate(
        [outs[c]["out"][:, 0:BLK].T.reshape(-1) for c in range(N_CORES)])
    e = np.concatenate(
        [outs[c]["out"][:, BLK:2 * BLK].T.reshape(-1) for c in range(N_CORES)])
    S_b = np.float32(sum(outs[c]["out2"][:, 0].sum(dtype=np.float64)
                         for c in range(N_CORES)))
    S_e = np.float32(sum(outs[c]["out2"][:, 1].sum(dtype=np.float64)
                         for c in range(N_CORES)))
    den = np.float32(S_b * S_e)

    idx, vals = _select_topk(b, e, den, k)
    return idx, vals


# revision 21
# speedup vs baseline: 1.0800x; 1.0800x over previous
"""Trainium2 Bass kernel for nn_Candidate_Scorer.

Reference computation:
    b = G_p @ wb            # [N,1]
    e = G_p @ we            # [N,1]
    num = exp(b + e.T)      # [N,N]
    den = sum(num)
    P = triu(num / den)
    top_k(P.reshape(-1), k) -> ((row, col) indices, values)

Key structure exploited:
  * num = exp(b) * exp(e).T is rank-1, so den = sum(exp(b)) * sum(exp(e)).
    No N x N reduction is needed.
  * exp is monotone, so the top-k of exp(b_i + e_j) over {j >= i} is the
    top-k of b_i + e_j over the same set -- selected from the two
    N-vectors with an exact thresholding argument (see _select_topk).

Device work (SPMD over 8 cores, rows sharded): b = G@wb, e = G@we as
batched multiply + reduce on the Vector engine (rows on partitions -
avoids the PE fp32 4-cycles/row penalty, keeps full f32 accuracy); the
weight vectors are broadcast across partitions by the otherwise-idle
TensorEngine (ones x w -> PSUM); exp on the Scalar engine; softmax-
denominator partials via a final Vector reduce.  Raw engine programs
(bacc) with manual semaphores; input G is pipelined over three DMA
channels (sync HWDGE, scalar HWDGE, gpsimd SWDGE) block by block so
compute starts after ~100 KB lands.
Host work (gather/merge): concatenate shards, exact top-k candidate
selection from the N-vectors, final value/index assembly.
"""

import numpy as np

N = 8192
D = 200
N_CORES = 8
ROWS = N // N_CORES    # 1024 rows per core
BLK = ROWS // 128      # 8 row-blocks of 128 partitions

_COMPILED = {}


def _build_program():
    """Per-core SPMD program (bacc, manual sync).

    Inputs (per core):
      "w"   [1, 400]   wb then we
      "g1"  [128, 400] G blocks 0-1 (row blk*128+p at partition p)
      "g2"  [128, 600] G blocks 2-4
      "g3"  [128, 600] G blocks 5-7
    Outputs (per core):
      "out"  [128, 16] cols v*8+blk = b (v=0) / e (v=1) values
      "out2" [128, 2]  per-partition sums of exp(b), exp(e)
    """
    import concourse.bass as bass
    import concourse.bacc as bacc
    import concourse.mybir as mybir

    dt = mybir.dt.float32
    fexp = mybir.ActivationFunctionType.Exp
    nc = bacc.Bacc("TRN2", target_bir_lowering=False, debug=False,
                   num_devices=N_CORES)

    g1_d = nc.dram_tensor("g1", [128, 4 * D], dt, kind="ExternalInput")
    g2_d = nc.dram_tensor("g2", [128, 3 * D], dt, kind="ExternalInput")
    g3_d = nc.dram_tensor("g3", [128, 3 * D], dt, kind="ExternalInput")
    out_d = nc.dram_tensor("out", [128, 16], dt, kind="ExternalOutput")
    out2_d = nc.dram_tensor("out2", [128, 2], dt, kind="ExternalOutput")

    with (
        nc.sbuf_tensor("gw_s", [128, (2 + BLK) * D], dt) as gw_s,
        nc.sbuf_tensor("out_s", [128, 16], dt) as out_s,
        nc.sbuf_tensor("out2_s", [128, 2], dt) as out2_s,
        nc.sbuf_tensor("prod_s", [128, 2 * 3 * D], dt) as prod_s,
        nc.sbuf_tensor("ebe_s", [128, 2 * BLK], dt) as ebe_s,
        nc.semaphore("s_r1") as s_r1,
        nc.semaphore("s_r2") as s_r2,
        nc.semaphore("s_r3") as s_r3,
        nc.semaphore("s_dve") as s_dve,
        nc.semaphore("s_act") as s_act,
        nc.semaphore("s_done") as s_done,
        nc.semaphore("s_out") as s_out,
        nc.Block() as block,
    ):
        # w broadcast view [128, 2, nb, 200] over cols 0:400
        def w4(nb):
            return (gw_s[:, 0:2 * D].rearrange("p (v d) -> p v d", v=2)
                    .rearrange("p v (z d) -> p v z d", z=1)
                    .broadcast_to((128, 2, nb, D)))

        # out columns viewed as [2, BLK]
        bev = out_s[:].rearrange("p (v z) -> p v z", v=2)

        @block.sync
        def _(sync):
            sync.dma_start(gw_s[:, 0:3 * D], g1_d[:, 0:3 * D]
                           ).then_inc(s_r1, 16)
            sync.dma_start(gw_s[:, 3 * D:4 * D], g1_d[:, 3 * D:]
                           ).then_inc(s_r1, 16)
            sync.wait_ge(s_dve, 1)
            sync.dma_start(out_d[:], out_s[:]).then_inc(s_out, 16)
            sync.wait_ge(s_done, 1)
            sync.dma_start(out2_d[:], out2_s[:]).then_inc(s_out, 16)
            sync.wait_ge(s_out, 32)

        @block.scalar
        def _(scalar):
            # blocks 2-4 on the ACT HWDGE ring, parallel with ring1
            scalar.dma_start(gw_s[:, 4 * D:7 * D], g2_d[:]).then_inc(s_r2, 16)
            # warm the Exp table while the DMAs fly (result discarded)
            nc.scalar.activation(ebe_s[:, 0:1], out_s[:, 0:1], fexp)
            scalar.wait_ge(s_dve, 1)
            nc.scalar.activation(ebe_s[:], out_s[:], fexp).then_inc(s_act, 1)

        @block.gpsimd
        def _(gpsimd):
            # blocks 5-7 via SWDGE, parallel with both HWDGE rings
            gpsimd.dma_start(gw_s[:, 7 * D:], g3_d[:]).then_inc(s_r3, 16)

        @block.vector
        def _(vector):
            # (z0, nb, [(sem, threshold), ...]) data gates per chunk
            plan = [(0, 1, [(s_r1, 16)]),
                    (1, 1, [(s_r1, 32)]),
                    (2, 3, [(s_r2, 16)]),
                    (5, 3, [(s_r3, 16)])]
            for z0, nb, gates in plan:
                for sem, thr in gates:
                    vector.wait_ge(sem, thr)
                c0 = (2 + z0) * D
                g4 = (gw_s[:, c0:c0 + nb * D]
                      .rearrange("p (z d) -> p z d", z=nb)
                      .rearrange("p z (u d) -> p u z d", u=1)
                      .broadcast_to((128, 2, nb, D)))
                p4 = (prod_s[:, 0:2 * nb * D]
                      .rearrange("p (v z d) -> p v z d", v=2, z=nb))
                nc.vector.tensor_tensor(p4, g4, w4(nb),
                                        op=mybir.AluOpType.mult)
                ins = nc.vector.reduce_sum(bev[:, :, z0:z0 + nb], p4,
                                           axis=mybir.AxisListType.X)
            ins.then_inc(s_dve, 1)
            # softmax-denominator partials: per-partition sums of exp
            vector.wait_ge(s_act, 1)
            e3 = ebe_s[:].rearrange("p (v z) -> p v z", v=2)
            nc.vector.reduce_sum(out2_s[:], e3, axis=mybir.AxisListType.X
                                 ).then_inc(s_done, 1)

    nc.compile()
    return nc


def _get_program():
    if "nc" not in _COMPILED:
        _COMPILED["nc"] = _build_program()
    return _COMPILED["nc"]


def _pack_inputs(G_p, wb, we):
    wb = wb.reshape(-1).astype(np.float32)
    we = we.reshape(-1).astype(np.float32)
    in_maps = []
    for c in range(N_CORES):
        shard = G_p[c * ROWS:(c + 1) * ROWS, :].astype(np.float32)
        blocks = shard.reshape(BLK, 128, D).transpose(1, 0, 2)  # [128,8,200]
        g1 = np.empty((128, 4 * D), dtype=np.float32)
        g1[:, 0:D] = wb[None, :]
        g1[:, D:2 * D] = we[None, :]
        g1[:, 2 * D:4 * D] = blocks[:, 0:2, :].reshape(128, 2 * D)
        in_maps.append({
            "g1": g1,
            "g2": np.ascontiguousarray(blocks[:, 2:5, :].reshape(128, 3 * D)),
            "g3": np.ascontiguousarray(blocks[:, 5:8, :].reshape(128, 3 * D)),
        })
    return in_maps


def _run_device(G_p, wb, we, trace=False):
    from concourse.bass_utils import run_bass_kernel_spmd

    nc = _get_program()
    in_maps = _pack_inputs(G_p, wb, we)
    res = run_bass_kernel_spmd(nc, in_maps, core_ids=list(range(N_CORES)),
                               trace=trace)
    return res


def _select_topk(b, e, den, k):
    """Exact top-k of exp(b_i + e_j)/den over {(i, j): j >= i}.

    Threshold argument: rowbest[i] = b[i] + max(e[i:]) is each row's best
    pair value. The k-th largest rowbest T is a lower bound on the k-th
    largest pair value (k distinct rows each contain a pair >= T), so
    every true top-k pair has value >= T. We enumerate all valid pairs
    with b_i + e_j >= T (minus a small safety margin) and rank them
    exactly as jax.lax.top_k does: by f32 value descending, ties broken
    by lower flat index.
    """
    bf = b.astype(np.float32)
    ef = e.astype(np.float32)
    n = bf.shape[0]

    suff = np.maximum.accumulate(ef[::-1])[::-1]   # suffix max of e
    rowbest = bf + suff
    kth = np.partition(rowbest, n - k)[n - k] - np.float32(1e-4)

    order_e = np.lexsort((np.arange(n), -ef))
    e_sorted = ef[order_e]

    rows = np.where(rowbest >= kth)[0]
    cand_i, cand_j = [], []
    for i in rows:
        t = kth - bf[i]
        cnt = int(np.searchsorted(-e_sorted, -t, side="right"))
        if cnt == 0:
            continue
        js = order_e[:cnt]
        js = js[js >= i]
        if js.size:
            cand_i.append(np.full(js.size, i, dtype=np.int64))
            cand_j.append(js)
    ci = np.concatenate(cand_i)
    cj = np.concatenate(cand_j)

    # values exactly as the reference computes them: f32 add, f32 exp,
    # f32 divide
    s = (bf[ci] + ef[cj]).astype(np.float32)
    v = np.exp(s).astype(np.float32) / np.float32(den)
    flat = ci * n + cj
    order = np.lexsort((flat, -v))[:k]
    top_i = ci[order]
    top_j = cj[order]
    idx = np.stack([top_i, top_j], axis=1).astype(np.int32)
    return idx, v[order].astype(np.float32)


def kernel(G_p, wb, we, k):
    G_p = np.asarray(G_p, dtype=np.float32)
    wb = np.asarray(wb, dtype=np.float32).reshape(D, 1)
    we = np.asarray(we, dtype=np.float32).reshape(D, 1)
    k = int(k)

    res = _run_device(G_p, wb, we)
    outs = res.results

    # out[:, v*8+blk] at partition p = b/e[blk*128 + p]
    b = np.concatenate(
        [outs[c]["out"][:, 0:BLK].T.reshape(-1) for c in range(N_CORES)])
    e = np.concatenate(
        [outs[c]["out"][:, BLK:2 * BLK].T.reshape(-1) for c in range(N_CORES)])
    S_b = np.float32(sum(outs[c]["out2"][:, 0].sum(dtype=np.float64)
                         for c in range(N_CORES)))
    S_e = np.float32(sum(outs[c]["out2"][:, 1].sum(dtype=np.float64)
                         for c in range(N_CORES)))
    den = np.float32(S_b * S_e)

    idx, vals = _select_topk(b, e, den, k)
    return idx, vals


# revision 22
# speedup vs baseline: 1.1644x; 1.0781x over previous
"""Trainium2 Bass kernel for nn_Candidate_Scorer.

Reference computation:
    b = G_p @ wb            # [N,1]
    e = G_p @ we            # [N,1]
    num = exp(b + e.T)      # [N,N]
    den = sum(num)
    P = triu(num / den)
    top_k(P.reshape(-1), k) -> ((row, col) indices, values)

Key structure exploited:
  * num = exp(b) * exp(e).T is rank-1, so den = sum(exp(b)) * sum(exp(e)).
    No N x N reduction is needed.
  * exp is monotone, so the top-k of exp(b_i + e_j) over {j >= i} is the
    top-k of b_i + e_j over the same set -- selected from the two
    N-vectors with an exact thresholding argument (see _select_topk).

Device work (SPMD over 8 cores, rows sharded): b = G@wb, e = G@we as
batched multiply + reduce on the Vector engine (rows on partitions -
avoids the PE fp32 4-cycles/row penalty, keeps full f32 accuracy), exp
and partial softmax-denominator sums on the Scalar engine.  Raw engine
programs (bacc) with manual semaphores; two parallel HWDGE input DMAs
(sync + scalar rings) overlapped with the first half of the compute.
Host work (gather/merge): concatenate shards, exact top-k candidate
selection from the N-vectors, final value/index assembly.
"""

import numpy as np

N = 8192
D = 200
N_CORES = 8
ROWS = N // N_CORES    # 1024 rows per core
BLK = ROWS // 128      # 8 row-blocks of 128 partitions
WCOLS = 2 * D          # wb & we broadcast region
GCOLS = BLK * D
# SBUF tile layout [128, 2000]:
#   cols 0:200    wb broadcast to all partitions
#   cols 200:400  we broadcast
#   cols 400+blk*200 : G row (blk*128+p, :) at partition p
# ring1 (sync HWDGE):   w + blocks 0-2   (cols    0:1000)
# ring2 (scalar HWDGE): blocks 3-7       (cols 1000:2000)
H1B = 3
RING1C = WCOLS + H1B * D
RING2C = WCOLS + GCOLS - RING1C

_COMPILED = {}


def _build_program():
    """Per-core SPMD program (bacc, manual sync).

    Inputs (per core):  "gw1" [128, 1000], "gw2" [128, 1000] f32
    Output (per core):  "out" [128, 18] f32:
      cols  0:8   b values   (b[blk*128 + p] at [p, blk])
      cols  8:16  e values
      col   16    per-partition sum of exp(b)
      col   17    per-partition sum of exp(e)
    """
    import concourse.bass as bass
    import concourse.bacc as bacc
    import concourse.mybir as mybir

    dt = mybir.dt.float32
    fexp = mybir.ActivationFunctionType.Exp
    nc = bacc.Bacc("TRN2", target_bir_lowering=False, debug=False,
                   num_devices=N_CORES)

    gw1_d = nc.dram_tensor("gw1", [128, RING1C], dt, kind="ExternalInput")
    gw2_d = nc.dram_tensor("gw2", [128, RING2C], dt, kind="ExternalInput")
    out_d = nc.dram_tensor("out", [128, 18], dt, kind="ExternalOutput")

    with (
        nc.sbuf_tensor("gw_s", [128, WCOLS + GCOLS], dt) as gw_s,
        nc.sbuf_tensor("out_s", [128, 18], dt) as out_s,
        nc.sbuf_tensor("prod_s", [128, 2 * (BLK - H1B) * D], dt) as prod_s,
        nc.sbuf_tensor("ebe_s", [128, 2 * BLK], dt) as ebe_s,
        nc.semaphore("s_r1") as s_r1,
        nc.semaphore("s_r2") as s_r2,
        nc.semaphore("s_dve") as s_dve,
        nc.semaphore("s_done") as s_done,
        nc.semaphore("s_out") as s_out,
        nc.Block() as block,
    ):
        def w4(nb):
            # w operand [128, 2, nb, 200]: v-axis strides between wb and
            # we, block axis is a stride-0 broadcast
            return (gw_s[:, 0:WCOLS]
                    .rearrange("p (v d) -> p v d", v=2)
                    .rearrange("p v (z d) -> p v z d", z=1)
                    .broadcast_to((128, 2, nb, D)))

        # out columns 0:16 viewed as [2, BLK]
        bev = out_s[:, 0:2 * BLK].rearrange("p (v z) -> p v z", v=2)
        halves = [(0, H1B), (H1B, BLK - H1B)]

        @block.sync
        def _(sync):
            sync.dma_start(gw_s[:, 0:RING1C], gw1_d[:]).then_inc(s_r1, 16)
            sync.wait_ge(s_done, 1)
            sync.dma_start(out_d[:], out_s[:]).then_inc(s_out, 16)
            sync.wait_ge(s_out, 16)

        @block.vector
        def _(vector):
            for half, (z0, nb) in enumerate(halves):
                vector.wait_ge(s_r1 if half == 0 else s_r2, 16)
                g0 = WCOLS + z0 * D
                g4 = (gw_s[:, g0:g0 + nb * D]
                      .rearrange("p (z d) -> p z d", z=nb)
                      .rearrange("p z (v d) -> p v z d", v=1)
                      .broadcast_to((128, 2, nb, D)))
                p4 = (prod_s[:, 0:2 * nb * D]
                      .rearrange("p (v z d) -> p v z d", v=2, z=nb))
                nc.vector.tensor_tensor(p4, g4, w4(nb),
                                        op=mybir.AluOpType.mult)
                ins = nc.vector.reduce_sum(
                    bev[:, :, z0:z0 + nb], p4,
                    axis=mybir.AxisListType.X)
            ins.then_inc(s_dve, 1)

        @block.scalar
        def _(scalar):
            # ring2 input DMA on the ACT HWDGE ring, parallel with ring1
            scalar.dma_start(gw_s[:, RING1C:], gw2_d[:]).then_inc(s_r2, 16)
            # warm the Exp table while the DMAs fly (result discarded)
            nc.scalar.activation(ebe_s[:, 0:1], out_s[:, 0:1], fexp)
            scalar.wait_ge(s_dve, 1)
            for v in range(2):
                ins = nc.scalar.activation(
                    ebe_s[:, v * BLK:(v + 1) * BLK],
                    out_s[:, v * BLK:(v + 1) * BLK],
                    fexp,
                    accum_out=out_s[:, 2 * BLK + v:2 * BLK + v + 1],
                )
            ins.then_inc(s_done, 1)

    nc.compile()
    return nc


def _get_program():
    if "nc" not in _COMPILED:
        _COMPILED["nc"] = _build_program()
    return _COMPILED["nc"]


def _pack_inputs(G_p, wb, we):
    wb = wb.reshape(-1).astype(np.float32)
    we = we.reshape(-1).astype(np.float32)
    in_maps = []
    for c in range(N_CORES):
        shard = G_p[c * ROWS:(c + 1) * ROWS, :].astype(np.float32)
        gw = np.empty((128, WCOLS + GCOLS), dtype=np.float32)
        gw[:, 0:D] = wb[None, :]
        gw[:, D:2 * D] = we[None, :]
        # blocks: partition p of block blk holds G row blk*128+p
        gw[:, WCOLS:] = shard.reshape(BLK, 128, D).transpose(1, 0, 2).reshape(
            128, GCOLS)
        in_maps.append({
            "gw1": np.ascontiguousarray(gw[:, 0:RING1C]),
            "gw2": np.ascontiguousarray(gw[:, RING1C:]),
        })
    return in_maps


def _run_device(G_p, wb, we, trace=False):
    from concourse.bass_utils import run_bass_kernel_spmd

    nc = _get_program()
    in_maps = _pack_inputs(G_p, wb, we)
    res = run_bass_kernel_spmd(nc, in_maps, core_ids=list(range(N_CORES)),
                               trace=trace)
    return res


def _select_topk(b, e, den, k):
    """Exact top-k of exp(b_i + e_j)/den over {(i, j): j >= i}.

    Threshold argument: rowbest[i] = b[i] + max(e[i:]) is each row's best
    pair value. The k-th largest rowbest T is a lower bound on the k-th
    largest pair value (k distinct rows each contain a pair >= T), so
    every true top-k pair has value >= T. We enumerate all valid pairs
    with b_i + e_j >= T (minus a small safety margin) and rank them
    exactly as jax.lax.top_k does: by f32 value descending, ties broken
    by lower flat index.
    """
    bf = b.astype(np.float32)
    ef = e.astype(np.float32)
    n = bf.shape[0]

    suff = np.maximum.accumulate(ef[::-1])[::-1]   # suffix max of e
    rowbest = bf + suff
    kth = np.partition(rowbest, n - k)[n - k] - np.float32(1e-4)

    order_e = np.lexsort((np.arange(n), -ef))
    e_sorted = ef[order_e]

    rows = np.where(rowbest >= kth)[0]
    cand_i, cand_j = [], []
    for i in rows:
        t = kth - bf[i]
        cnt = int(np.searchsorted(-e_sorted, -t, side="right"))
        if cnt == 0:
            continue
        js = order_e[:cnt]
        js = js[js >= i]
        if js.size:
            cand_i.append(np.full(js.size, i, dtype=np.int64))
            cand_j.append(js)
    ci = np.concatenate(cand_i)
    cj = np.concatenate(cand_j)

    # values exactly as the reference computes them: f32 add, f32 exp,
    # f32 divide
    s = (bf[ci] + ef[cj]).astype(np.float32)
    v = np.exp(s).astype(np.float32) / np.float32(den)
    flat = ci * n + cj
    order = np.lexsort((flat, -v))[:k]
    top_i = ci[order]
    top_j = cj[order]
    idx = np.stack([top_i, top_j], axis=1).astype(np.int32)
    return idx, v[order].astype(np.float32)


def kernel(G_p, wb, we, k):
    G_p = np.asarray(G_p, dtype=np.float32)
    wb = np.asarray(wb, dtype=np.float32).reshape(D, 1)
    we = np.asarray(we, dtype=np.float32).reshape(D, 1)
    k = int(k)

    res = _run_device(G_p, wb, we)
    outs = res.results

    # out[:, v*8+blk] at partition p = b/e[blk*128 + p]
    b = np.concatenate(
        [outs[c]["out"][:, 0:BLK].T.reshape(-1) for c in range(N_CORES)])
    e = np.concatenate(
        [outs[c]["out"][:, BLK:2 * BLK].T.reshape(-1) for c in range(N_CORES)])
    S_b = np.float32(sum(outs[c]["out"][:, 2 * BLK].sum(dtype=np.float64)
                         for c in range(N_CORES)))
    S_e = np.float32(sum(outs[c]["out"][:, 2 * BLK + 1].sum(dtype=np.float64)
                         for c in range(N_CORES)))
    den = np.float32(S_b * S_e)

    idx, vals = _select_topk(b, e, den, k)
    return idx, vals
